# revision 20
# baseline (speedup 1.0000x reference)
"""AttentionSuper (AutoFormer relative-position attention) on 8 trn2 cores.

Data-parallel over batch B=64 -> 8 batches/core (BH=80 fused (b,h) rows).
Per core, attention runs in TRANSPOSED score layout attnT[j, i] per (b,h):
  - value matmuls use attnT as lhsT directly, and softmax normalization
    comes free from a ones-column appended to v.
  - the rel-pos key bias is folded in multiplicatively:
      exp(scale*(s+b)) = exp(scale*s) * exp(scale*b)
    Phase A computes expB[j,i,bh] = exp(scale * q_i . rel_k[i,j]) for all
    (b,h) at once per i (PE matmul, ACT exp-evacuation from PSUM), and
    Phase B multiplies it into exp(scale*s) on the DVE (bf16 2x mode).
Everything is bf16 except PSUM accumulation (f32); scores are bounded
(|scale*s| < ~6) so softmax skips max-subtraction.

v2 rewrite targets instruction-count bottlenecks found in the v1 trace:
fp32 score matmuls (4x slower than bf16), 766 per-tensor dma_starts
(~600ns each on the Sync engine), and per-(bh,half) vector ops.
"""

import sys

import numpy as np

sys.path.insert(0, "/opt/trn_rl_repo")

import ml_dtypes  # noqa: E402

B, N, H, D = 64, 197, 10, 64
MAX_REL = 14
NCORES = 8
BSH = B // NCORES          # batches per core
BH = BSH * H               # 80 fused (batch, head) rows per core
P2 = N - 128               # 69
NP = N + 1                 # 198: padded i-stride (keeps bf16 rows 4B-aligned)
SCALE = D ** (-0.5)

_bf16 = ml_dtypes.bfloat16

LAST_EXEC_NS = None
LAST_TRACE = None
_CACHED = None


def _rel_indices():
    s = int(np.sqrt(N))
    r = np.arange(N)
    dist_v = r[None, :] // s - r[:, None] // s
    dist_h = r[None, :] % s - r[:, None] % s
    iv = np.clip(dist_v, -MAX_REL, MAX_REL) + MAX_REL + 1
    ih = np.clip(dist_h, -MAX_REL, MAX_REL) + MAX_REL + 1
    iv = np.pad(iv[1:, 1:], ((1, 0), (1, 0)), constant_values=0)
    ih = np.pad(ih[1:, 1:], ((1, 0), (1, 0)), constant_values=0)
    return iv, ih


def _build_module():
    import concourse.bass as bass  # noqa: F401
    import concourse.bacc as bacc
    import concourse.tile as tile
    from concourse import mybir

    f32 = mybir.dt.float32
    bf16 = mybir.dt.bfloat16
    Exp = mybir.ActivationFunctionType.Exp

    nc = bacc.Bacc()

    qT2 = nc.dram_tensor("qT2", [D, BH, NP], bf16, kind="ExternalInput")
    kT2 = nc.dram_tensor("kT2", [D, BH, NP], bf16, kind="ExternalInput")
    vb2 = nc.dram_tensor("vb2", [N, BH, D], bf16, kind="ExternalInput")
    qTi = nc.dram_tensor("qTi", [D, N, BH], bf16, kind="ExternalInput")
    rkT = nc.dram_tensor("rkT", [D, N, NP], bf16, kind="ExternalInput")
    rv = nc.dram_tensor("rv", [N, N, D], bf16, kind="ExternalInput")
    out = nc.dram_tensor("out", [BSH, N, H * D], bf16, kind="ExternalOutput")
    # rel-v partials, split lo/hi so Phase C's lo half only depends on the
    # first 8 B2 chunks (name-based dep tracking) and overlaps B2's tail
    o2d_lo = nc.dram_tensor("o2d_lo", [128, BH, D], bf16)
    o2d_hi = nc.dram_tensor("o2d_hi", [P2, BH, D], bf16)

    with tile.TileContext(nc) as tc:
        with tc.tile_pool(name="persist", bufs=1) as persist:
            # attnT[j, bh, i] (i-stride NP for alignment; col N is garbage)
            attnT_lo = persist.tile([128, BH, NP], bf16)
            attnT_hi = persist.tile([128, BH, NP], bf16)   # j in [128,197) on parts 0..68
            # raw O1 + denominator: [i, bh, 66] (col 64 = sum, col 65 = 0)
            o1sb_lo = persist.tile([128, BH, 66], bf16)
            o1sb_hi = persist.tile([128, BH, 66], bf16)
            recips = persist.tile([128, 2 * BH], f32)

            with tc.tile_pool(name="expb", bufs=1) as expbp:
                expB_lo = expbp.tile([128, BH, NP], bf16)
                expB_hi = expbp.tile([128, BH, NP], bf16)

                # ---- Phase A: expB[j, bh, i] = exp(scale * sum_d rel_k[i,j,d] q[bh,i,d])
                CH = 12
                with (
                    tc.tile_pool(name="pa", bufs=3) as pa,
                    tc.tile_pool(name="pap", bufs=2, space="PSUM") as pap,
                ):
                    for c0 in range(0, N, CH):
                        cn = min(CH, N - c0)
                        rk_t = pa.tile([D, CH, NP], bf16, tag="rk")
                        nc.sync.dma_start(rk_t[:, :cn, :], rkT[:, c0 : c0 + cn, :])
                        qb_t = pa.tile([D, CH, BH], bf16, tag="qb")
                        nc.sync.dma_start(qb_t[:, :cn, :], qTi[:, c0 : c0 + cn, :])
                        ps_lo = pap.tile([128, 1024], f32, tag="pslo")
                        ps_hi = pap.tile([128, 1024], f32, tag="pshi")
                        for k in range(cn):
                            off = (k // 6) * 512 + (k % 6) * 80
                            nc.tensor.matmul(
                                ps_lo[:, off : off + 80],
                                rk_t[:, k, 0:128], qb_t[:, k, :],
                                start=True, stop=True,
                            )
                            nc.tensor.matmul(
                                ps_hi[0:P2, off : off + 80],
                                rk_t[:, k, 128:N], qb_t[:, k, :],
                                start=True, stop=True,
                            )
                        for ps, dst, npart in (
                            (ps_lo, expB_lo, 128), (ps_hi, expB_hi, P2)
                        ):
                            if cn == CH:
                                src = (
                                    ps[:]
                                    .rearrange("p (b r) -> p b r", b=2)[:, :, 0:480]
                                    .rearrange("p b (k e) -> p e b k", e=80)
                                )
                                d = dst[0:npart, :, c0 : c0 + cn].rearrange(
                                    "p h (b k) -> p h b k", b=2
                                )
                                nc.scalar.activation(
                                    d, src[0:npart], Exp, scale=SCALE
                                )
                            else:
                                src = ps[:, 0 : cn * 80].rearrange(
                                    "p (k e) -> p e k", e=80
                                )
                                nc.scalar.activation(
                                    dst[0:npart, :, c0 : c0 + cn],
                                    src[0:npart], Exp, scale=SCALE,
                                )

                # ---- Phase B1: per (b,h) pair: scoresT, exp, *expB, O1 = attnT.T @ [v|1]
                GB = 16
                with (
                    tc.tile_pool(name="pb", bufs=2) as pb,
                    tc.tile_pool(name="pbe", bufs=2) as pbe,
                    tc.tile_pool(name="pbp", bufs=2, space="PSUM") as pbp,
                ):
                    for g0 in range(0, BH, GB):
                        q2 = pb.tile([D, GB, NP], bf16, tag="q2")
                        nc.sync.dma_start(q2[:], qT2[:, g0 : g0 + GB, :])
                        k2 = pb.tile([D, GB, NP], bf16, tag="k2")
                        nc.sync.dma_start(k2[:], kT2[:, g0 : g0 + GB, :])
                        v2l = pb.tile([128, GB, 66], bf16, tag="v2l")
                        nc.sync.dma_start(
                            v2l[:, :, 0:64], vb2[0:128, g0 : g0 + GB, :]
                        )
                        nc.vector.memset(v2l[:, :, 64:65], 1.0)
                        nc.vector.memset(v2l[:, :, 65:66], 0.0)
                        v2h = pb.tile([128, GB, 66], bf16, tag="v2h")
                        nc.sync.dma_start(
                            v2h[0:P2, :, 0:64], vb2[128:N, g0 : g0 + GB, :]
                        )
                        nc.vector.memset(v2h[0:P2, :, 64:65], 1.0)
                        nc.vector.memset(v2h[0:P2, :, 65:66], 0.0)

                        for p0 in range(0, GB, 2):
                            bh = g0 + p0
                            ps_lo = pbp.tile([128, 2, NP], f32, tag="pslo")
                            ps_hi = pbp.tile([128, 2, NP], f32, tag="pshi")
                            for pp in range(2):
                                nc.tensor.matmul(
                                    ps_lo[:, pp, 0:197],
                                    k2[:, p0 + pp, 0:128], q2[:, p0 + pp, 0:N],
                                    start=True, stop=True,
                                )
                                nc.tensor.matmul(
                                    ps_hi[0:P2, pp, 0:197],
                                    k2[:, p0 + pp, 128:N], q2[:, p0 + pp, 0:N],
                                    start=True, stop=True,
                                )
                            es_lo = pbe.tile([128, 2, NP], bf16, tag="eslo")
                            nc.scalar.activation(
                                es_lo[:, :, 0:197], ps_lo[:, :, 0:197],
                                Exp, scale=SCALE,
                            )
                            es_hi = pbe.tile([128, 2, NP], bf16, tag="eshi")
                            nc.scalar.activation(
                                es_hi[0:P2, :, 0:197], ps_hi[0:P2, :, 0:197],
                                Exp, scale=SCALE,
                            )
                            nc.vector.tensor_mul(
                                attnT_lo[:, bh : bh + 2, :],
                                es_lo[:], expB_lo[:, bh : bh + 2, :],
                            )
                            nc.vector.tensor_mul(
                                attnT_hi[0:P2, bh : bh + 2, :],
                                es_hi[0:P2], expB_hi[0:P2, bh : bh + 2, :],
                            )

                            o1_lo = pbp.tile([128, 2, 66], f32, tag="o1lo")
                            o1_hi = pbp.tile([128, 2, 66], f32, tag="o1hi")
                            for pp in range(2):
                                nc.tensor.matmul(
                                    o1_lo[:, pp, :],
                                    attnT_lo[:, bh + pp, 0:128],
                                    v2l[:, p0 + pp, :],
                                    start=True, stop=False,
                                )
                                nc.tensor.matmul(
                                    o1_lo[:, pp, :],
                                    attnT_hi[0:P2, bh + pp, 0:128],
                                    v2h[0:P2, p0 + pp, :],
                                    start=False, stop=True,
                                )
                                nc.tensor.matmul(
                                    o1_hi[0:P2, pp, :],
                                    attnT_lo[:, bh + pp, 128:197],
                                    v2l[:, p0 + pp, :],
                                    start=True, stop=False,
                                )
                                nc.tensor.matmul(
                                    o1_hi[0:P2, pp, :],
                                    attnT_hi[0:P2, bh + pp, 128:197],
                                    v2h[0:P2, p0 + pp, :],
                                    start=False, stop=True,
                                )
                            nc.vector.tensor_copy(
                                o1sb_lo[:, bh : bh + 2, :], o1_lo[:]
                            )
                            nc.vector.tensor_copy(
                                o1sb_hi[0:P2, bh : bh + 2, :], o1_hi[0:P2]
                            )

                    nc.vector.reciprocal(
                        recips[:, 0:BH], o1sb_lo[:, :, 64:65].squeeze(2)
                    )
                    nc.vector.reciprocal(
                        recips[0:P2, BH : 2 * BH],
                        o1sb_hi[0:P2, :, 64:65].squeeze(2),
                    )

            # ---- Phase B2: O2[i, bh, d] = sum_j attnT[j, bh, i] rel_v[i, j, d]
            CH2 = 16
            with (
                tc.tile_pool(name="pc", bufs=3) as pc,
                tc.tile_pool(name="pcp", bufs=3, space="PSUM") as pcp,
            ):
                nb2 = 0
                for c0 in range(0, N, CH2):
                    cn = min(CH2, N - c0)
                    rvl = pc.tile([128, CH2, D], bf16, tag="rvl")
                    nc.sync.dma_start(rvl[:, :cn, :], rv[0:128, c0 : c0 + cn, :])
                    rvh = pc.tile([128, CH2, D], bf16, tag="rvh")
                    nc.sync.dma_start(rvh[0:P2, :cn, :], rv[128:N, c0 : c0 + cn, :])
                    o2s = pc.tile([BH, CH2, D], bf16, tag="o2s")
                    for hb in range(0, cn, 8):
                        gn = min(8, cn - hb)
                        o2 = pcp.tile([BH, 512], f32, tag="o2")
                        for k in range(gn):
                            i = c0 + hb + k
                            nc.tensor.matmul(
                                o2[:, k * 64 : (k + 1) * 64],
                                attnT_lo[:, :, i], rvl[:, hb + k, :],
                                start=True, stop=False,
                            )
                            nc.tensor.matmul(
                                o2[:, k * 64 : (k + 1) * 64],
                                attnT_hi[0:P2, :, i], rvh[0:P2, hb + k, :],
                                start=False, stop=True,
                            )
                        if nb2 % 2 == 0:
                            nc.vector.tensor_copy(
                                o2s[:, hb : hb + gn, :], o2[:, 0 : gn * 64]
                            )
                        else:
                            nc.scalar.copy(
                                o2s[:, hb : hb + gn, :], o2[:, 0 : gn * 64]
                            )
                        nb2 += 1
                    # issue on ACT's DGE queue: a store blocked on the evac
                    # sem must not stall SP's rv prefetch stream
                    if c0 < 128:
                        dst = o2d_lo[c0 : c0 + cn, :, :]
                    else:
                        dst = o2d_hi[c0 - 128 : c0 - 128 + cn, :, :]
                    nc.scalar.dma_start(dst.transpose([1, 0, 2]), o2s[:, :cn, :])

            # ---- Phase C: out[b, i, (h d)] = (O1raw + O2) * recip
            with tc.tile_pool(name="pd", bufs=1) as pd:
                t2s = {}
                for src, cn, hx in ((o2d_lo, 128, 0), (o2d_hi, P2, 1)):
                    t2s[hx] = pd.tile(
                        [128, BH, D], bf16, tag=f"t2{hx}", name=f"t2{hx}"
                    )
                    # gpsimd (software DGE) queue: its dep-wait on the o2d
                    # stores must not block SP's rv-load stream for B2's tail
                    nc.gpsimd.dma_start(t2s[hx][0:cn], src[:])
                for c0, cn, o1sb, rcol, hx in (
                    (0, 128, o1sb_lo, 0, 0), (128, P2, o1sb_hi, BH, 1),
                ):
                    t2 = t2s[hx]
                    addf = pd.tile([128, BH, D], bf16, tag=f"addf{hx}")
                    nc.vector.tensor_add(
                        addf[0:cn], t2[0:cn], o1sb[0:cn, :, 0:64]
                    )
                    res = pd.tile([128, BH, D], bf16, tag=f"res{hx}")
                    for bh in range(BH):
                        sc = recips[0:cn, rcol + bh : rcol + bh + 1]
                        if bh % 3 != 2:
                            nc.vector.tensor_scalar_mul(
                                res[0:cn, bh, :], addf[0:cn, bh, :], sc
                            )
                        else:
                            nc.scalar.mul(res[0:cn, bh, :], addf[0:cn, bh, :], sc)
                    for b in range(BSH):
                        nc.sync.dma_start(
                            out[b, c0 : c0 + cn, :],
                            res[0:cn, b * H : (b + 1) * H, :],
                        )

    nc.finalize()
    return nc


def _get_module():
    global _CACHED
    if _CACHED is None:
        _CACHED = _build_module()
    return _CACHED


def kernel(x, k_table_v, k_table_h, v_table_v, v_table_h, _trace=False):
    global LAST_EXEC_NS, LAST_TRACE
    from concourse.bass_utils import run_bass_kernel_spmd

    x = np.asarray(x, dtype=np.float32)
    iv, ih = _rel_indices()
    rel_k = np.asarray(k_table_v)[iv] + np.asarray(k_table_h)[ih]  # [N,N,D]
    rel_v = np.asarray(v_table_v)[iv] + np.asarray(v_table_h)[ih]  # [N,N,D]

    qkv = x.reshape(B, N, 3, H, D).transpose(2, 0, 3, 1, 4)  # [3,B,H,N,D]
    q, k, v = qkv[0], qkv[1], qkv[2]  # [B,H,N,D]

    rkT_host = np.zeros((D, N, NP), dtype=_bf16)  # [D,N(i),N(j)] j-padded
    rkT_host[:, :, :N] = rel_k.transpose(2, 0, 1).astype(_bf16)
    rv_host = np.ascontiguousarray(
        rel_v.transpose(1, 0, 2).astype(_bf16)
    )  # [N(j),N(i),D]

    def _padN(a):  # [D,BH,N] -> [D,BH,NP]
        out_ = np.zeros((D, BH, NP), dtype=_bf16)
        out_[:, :, :N] = a
        return out_

    in_maps = []
    for c in range(NCORES):
        qs = q[c * BSH : (c + 1) * BSH].reshape(BH, N, D)   # [BH,N,D]
        ks = k[c * BSH : (c + 1) * BSH].reshape(BH, N, D)
        vs = v[c * BSH : (c + 1) * BSH].reshape(BH, N, D)
        in_maps.append(
            {
                "qT2": _padN(qs.transpose(2, 0, 1).astype(_bf16)),  # [D,BH,NP]
                "kT2": _padN(ks.transpose(2, 0, 1).astype(_bf16)),
                "vb2": np.ascontiguousarray(
                    vs.transpose(1, 0, 2).astype(_bf16)
                ),  # [N,BH,D]
                "qTi": np.ascontiguousarray(
                    qs.transpose(2, 1, 0).astype(_bf16)
                ),  # [D,N,BH]
                "rkT": rkT_host,
                "rv": rv_host,
            }
        )

    nc = _get_module()
    res = run_bass_kernel_spmd(
        nc, in_maps, core_ids=list(range(NCORES)), trace=_trace
    )
    LAST_EXEC_NS = res.exec_time_ns
    LAST_TRACE = res.instructions_and_trace
    outs = [res.results[c]["out"].astype(np.float32) for c in range(NCORES)]
    return np.concatenate(outs, axis=0)


# revision 22
# speedup vs baseline: 1.1625x; 1.1625x over previous
"""AttentionSuper (AutoFormer relative-position attention) on 8 trn2 cores.

Data-parallel over batch B=64 -> 8 batches/core (BH=80 fused (b,h) rows).
Per core, attention runs in TRANSPOSED score layout attnT[j, i] per (b,h):
  - value matmuls use attnT as lhsT directly, and softmax normalization
    comes free from a ones-column appended to v.
  - the rel-pos key bias is folded in multiplicatively:
      exp(scale*(s+b)) = exp(scale*s) * exp(scale*b)
    Phase A computes expB[j,i,bh] = exp(scale * q_i . rel_k[i,j]) for all
    (b,h) at once per i (PE matmul, ACT exp-evacuation from PSUM), and
    Phase B multiplies it into exp(scale*s) on the DVE (bf16 2x mode).
Everything is bf16 except PSUM accumulation (f32); scores are bounded
(|scale*s| < ~6) so softmax skips max-subtraction.

v2 rewrite targets instruction-count bottlenecks found in the v1 trace:
fp32 score matmuls (4x slower than bf16), 766 per-tensor dma_starts
(~600ns each on the Sync engine), and per-(bh,half) vector ops.
"""

import sys

import numpy as np

sys.path.insert(0, "/opt/trn_rl_repo")

import ml_dtypes  # noqa: E402

B, N, H, D = 64, 197, 10, 64
MAX_REL = 14
NCORES = 8
BSH = B // NCORES          # batches per core
BH = BSH * H               # 80 fused (batch, head) rows per core
P2 = N - 128               # 69
NP = N + 1                 # 198: padded i-stride (keeps bf16 rows 4B-aligned)
SCALE = D ** (-0.5)

_bf16 = ml_dtypes.bfloat16

LAST_EXEC_NS = None
LAST_TRACE = None
_CACHED = None


def _rel_indices():
    s = int(np.sqrt(N))
    r = np.arange(N)
    dist_v = r[None, :] // s - r[:, None] // s
    dist_h = r[None, :] % s - r[:, None] % s
    iv = np.clip(dist_v, -MAX_REL, MAX_REL) + MAX_REL + 1
    ih = np.clip(dist_h, -MAX_REL, MAX_REL) + MAX_REL + 1
    iv = np.pad(iv[1:, 1:], ((1, 0), (1, 0)), constant_values=0)
    ih = np.pad(ih[1:, 1:], ((1, 0), (1, 0)), constant_values=0)
    return iv, ih


def _build_module():
    import concourse.bass as bass  # noqa: F401
    import concourse.bacc as bacc
    import concourse.tile as tile
    from concourse import mybir

    f32 = mybir.dt.float32
    bf16 = mybir.dt.bfloat16
    Exp = mybir.ActivationFunctionType.Exp

    nc = bacc.Bacc()

    qT2 = nc.dram_tensor("qT2", [D, BH, NP], bf16, kind="ExternalInput")
    kT2 = nc.dram_tensor("kT2", [D, BH, NP], bf16, kind="ExternalInput")
    vb2 = nc.dram_tensor("vb2", [N, BH, D], bf16, kind="ExternalInput")
    qTi = nc.dram_tensor("qTi", [D, N, BH], bf16, kind="ExternalInput")
    rkT = nc.dram_tensor("rkT", [D, N, NP], bf16, kind="ExternalInput")
    rv = nc.dram_tensor("rv", [N, N, D], bf16, kind="ExternalInput")
    out = nc.dram_tensor("out", [BSH, N, H * D], bf16, kind="ExternalOutput")
    # rel-v partials, split lo/hi so Phase C's lo half only depends on the
    # first 8 B2 chunks (name-based dep tracking) and overlaps B2's tail
    o2d_lo = nc.dram_tensor("o2d_lo", [128, BH, D], bf16)
    o2d_hi = nc.dram_tensor("o2d_hi", [P2, BH, D], bf16)

    with tile.TileContext(nc) as tc:
        with tc.tile_pool(name="persist", bufs=1) as persist:
            # attnT[j, bh, i] (i-stride NP for alignment; col N is garbage)
            attnT_lo = persist.tile([128, BH, NP], bf16)
            attnT_hi = persist.tile([128, BH, NP], bf16)   # j in [128,197) on parts 0..68
            # raw O1 + denominator: [i, bh, 66] (col 64 = sum, col 65 = 0)
            o1sb_lo = persist.tile([128, BH, 66], bf16)
            o1sb_hi = persist.tile([128, BH, 66], bf16)
            recips = persist.tile([128, 2 * BH], f32)

            with tc.tile_pool(name="expb", bufs=1) as expbp:
                expB_lo = expbp.tile([128, BH, NP], bf16)
                expB_hi = expbp.tile([128, BH, NP], bf16)

                # ---- Phase A: expB[j, bh, i] = exp(scale * sum_d rel_k[i,j,d] q[bh,i,d])
                CH = 12
                with (
                    tc.tile_pool(name="pa", bufs=3) as pa,
                    tc.tile_pool(name="pap", bufs=2, space="PSUM") as pap,
                ):
                    for c0 in range(0, N, CH):
                        cn = min(CH, N - c0)
                        rk_t = pa.tile([D, CH, NP], bf16, tag="rk")
                        nc.sync.dma_start(rk_t[:, :cn, :], rkT[:, c0 : c0 + cn, :])
                        qb_t = pa.tile([D, CH, BH], bf16, tag="qb")
                        nc.sync.dma_start(qb_t[:, :cn, :], qTi[:, c0 : c0 + cn, :])
                        ps_lo = pap.tile([128, 1024], f32, tag="pslo")
                        ps_hi = pap.tile([128, 1024], f32, tag="pshi")
                        for k in range(cn):
                            off = (k // 6) * 512 + (k % 6) * 80
                            nc.tensor.matmul(
                                ps_lo[:, off : off + 80],
                                rk_t[:, k, 0:128], qb_t[:, k, :],
                                start=True, stop=True,
                            )
                            nc.tensor.matmul(
                                ps_hi[0:P2, off : off + 80],
                                rk_t[:, k, 128:N], qb_t[:, k, :],
                                start=True, stop=True,
                            )
                        for ps, dst, npart in (
                            (ps_lo, expB_lo, 128), (ps_hi, expB_hi, P2)
                        ):
                            if cn == CH:
                                src = (
                                    ps[:]
                                    .rearrange("p (b r) -> p b r", b=2)[:, :, 0:480]
                                    .rearrange("p b (k e) -> p e b k", e=80)
                                )
                                d = dst[0:npart, :, c0 : c0 + cn].rearrange(
                                    "p h (b k) -> p h b k", b=2
                                )
                                nc.scalar.activation(
                                    d, src[0:npart], Exp, scale=SCALE
                                )
                            else:
                                src = ps[:, 0 : cn * 80].rearrange(
                                    "p (k e) -> p e k", e=80
                                )
                                nc.scalar.activation(
                                    dst[0:npart, :, c0 : c0 + cn],
                                    src[0:npart], Exp, scale=SCALE,
                                )

                # ---- Phase B1: per (b,h) pair: scoresT, exp, *expB, O1 = attnT.T @ [v|1]
                GB = 16
                with (
                    tc.tile_pool(name="pb", bufs=2) as pb,
                    tc.tile_pool(name="pbe", bufs=2) as pbe,
                    tc.tile_pool(name="pbp", bufs=2, space="PSUM") as pbp,
                ):
                    for g0 in range(0, BH, GB):
                        q2 = pb.tile([D, GB, NP], bf16, tag="q2")
                        nc.sync.dma_start(q2[:], qT2[:, g0 : g0 + GB, :])
                        k2 = pb.tile([D, GB, NP], bf16, tag="k2")
                        nc.sync.dma_start(k2[:], kT2[:, g0 : g0 + GB, :])
                        v2l = pb.tile([128, GB, 66], bf16, tag="v2l")
                        nc.sync.dma_start(
                            v2l[:, :, 0:64], vb2[0:128, g0 : g0 + GB, :]
                        )
                        nc.vector.memset(v2l[:, :, 64:65], 1.0)
                        nc.vector.memset(v2l[:, :, 65:66], 0.0)
                        v2h = pb.tile([128, GB, 66], bf16, tag="v2h")
                        nc.sync.dma_start(
                            v2h[0:P2, :, 0:64], vb2[128:N, g0 : g0 + GB, :]
                        )
                        nc.vector.memset(v2h[0:P2, :, 64:65], 1.0)
                        nc.vector.memset(v2h[0:P2, :, 65:66], 0.0)

                        for p0 in range(0, GB, 2):
                            bh = g0 + p0
                            ps_lo = pbp.tile([128, 2, NP], f32, tag="pslo")
                            ps_hi = pbp.tile([128, 2, NP], f32, tag="pshi")
                            for pp in range(2):
                                nc.tensor.matmul(
                                    ps_lo[:, pp, 0:197],
                                    k2[:, p0 + pp, 0:128], q2[:, p0 + pp, 0:N],
                                    start=True, stop=True,
                                )
                                nc.tensor.matmul(
                                    ps_hi[0:P2, pp, 0:197],
                                    k2[:, p0 + pp, 128:N], q2[:, p0 + pp, 0:N],
                                    start=True, stop=True,
                                )
                            es_lo = pbe.tile([128, 2, NP], bf16, tag="eslo")
                            nc.scalar.activation(
                                es_lo[:, :, 0:197], ps_lo[:, :, 0:197],
                                Exp, scale=SCALE,
                            )
                            es_hi = pbe.tile([128, 2, NP], bf16, tag="eshi")
                            nc.scalar.activation(
                                es_hi[0:P2, :, 0:197], ps_hi[0:P2, :, 0:197],
                                Exp, scale=SCALE,
                            )
                            nc.vector.tensor_mul(
                                attnT_lo[:, bh : bh + 2, :],
                                es_lo[:], expB_lo[:, bh : bh + 2, :],
                            )
                            nc.vector.tensor_mul(
                                attnT_hi[0:P2, bh : bh + 2, :],
                                es_hi[0:P2], expB_hi[0:P2, bh : bh + 2, :],
                            )

                            o1_lo = pbp.tile([128, 2, 66], f32, tag="o1lo")
                            o1_hi = pbp.tile([128, 2, 66], f32, tag="o1hi")
                            for pp in range(2):
                                nc.tensor.matmul(
                                    o1_lo[:, pp, :],
                                    attnT_lo[:, bh + pp, 0:128],
                                    v2l[:, p0 + pp, :],
                                    start=True, stop=False,
                                )
                                nc.tensor.matmul(
                                    o1_lo[:, pp, :],
                                    attnT_hi[0:P2, bh + pp, 0:128],
                                    v2h[0:P2, p0 + pp, :],
                                    start=False, stop=True,
                                )
                                nc.tensor.matmul(
                                    o1_hi[0:P2, pp, :],
                                    attnT_lo[:, bh + pp, 128:197],
                                    v2l[:, p0 + pp, :],
                                    start=True, stop=False,
                                )
                                nc.tensor.matmul(
                                    o1_hi[0:P2, pp, :],
                                    attnT_hi[0:P2, bh + pp, 128:197],
                                    v2h[0:P2, p0 + pp, :],
                                    start=False, stop=True,
                                )
                            nc.vector.tensor_copy(
                                o1sb_lo[:, bh : bh + 2, :], o1_lo[:]
                            )
                            nc.vector.tensor_copy(
                                o1sb_hi[0:P2, bh : bh + 2, :], o1_hi[0:P2]
                            )

                    nc.vector.reciprocal(
                        recips[:, 0:BH], o1sb_lo[:, :, 64:65].squeeze(2)
                    )
                    nc.vector.reciprocal(
                        recips[0:P2, BH : 2 * BH],
                        o1sb_hi[0:P2, :, 64:65].squeeze(2),
                    )

            # ---- Phase B2: O2[i, bh, d] = sum_j attnT[j, bh, i] rel_v[i, j, d]
            CH2 = 16
            with (
                tc.tile_pool(name="pc", bufs=3) as pc,
                tc.tile_pool(name="pcp", bufs=3, space="PSUM") as pcp,
            ):
                nb2 = 0
                for c0 in range(0, N, CH2):
                    cn = min(CH2, N - c0)
                    rvl = pc.tile([128, CH2, D], bf16, tag="rvl")
                    nc.sync.dma_start(rvl[:, :cn, :], rv[0:128, c0 : c0 + cn, :])
                    rvh = pc.tile([128, CH2, D], bf16, tag="rvh")
                    nc.sync.dma_start(rvh[0:P2, :cn, :], rv[128:N, c0 : c0 + cn, :])
                    o2s = pc.tile([BH, CH2, D], bf16, tag="o2s")
                    for hb in range(0, cn, 8):
                        gn = min(8, cn - hb)
                        o2 = pcp.tile([BH, 512], f32, tag="o2")
                        for k in range(gn):
                            i = c0 + hb + k
                            nc.tensor.matmul(
                                o2[:, k * 64 : (k + 1) * 64],
                                attnT_lo[:, :, i], rvl[:, hb + k, :],
                                start=True, stop=False,
                            )
                            nc.tensor.matmul(
                                o2[:, k * 64 : (k + 1) * 64],
                                attnT_hi[0:P2, :, i], rvh[0:P2, hb + k, :],
                                start=False, stop=True,
                            )
                        if nb2 % 2 == 0:
                            nc.vector.tensor_copy(
                                o2s[:, hb : hb + gn, :], o2[:, 0 : gn * 64]
                            )
                        else:
                            nc.scalar.copy(
                                o2s[:, hb : hb + gn, :], o2[:, 0 : gn * 64]
                            )
                        nb2 += 1
                    # issue on ACT's DGE queue: a store blocked on the evac
                    # sem must not stall SP's rv prefetch stream
                    if c0 < 128:
                        dst = o2d_lo[c0 : c0 + cn, :, :]
                    else:
                        dst = o2d_hi[c0 - 128 : c0 - 128 + cn, :, :]
                    nc.scalar.dma_start(dst.transpose([1, 0, 2]), o2s[:, :cn, :])

            tc.strict_bb_all_engine_barrier()

            # ---- Phase C: out[b, i, (h d)] = (O1raw + O2) * recip
            with tc.tile_pool(name="pd", bufs=1) as pd:
                t2s = {}
                for src, cn, hx in ((o2d_lo, 128, 0), (o2d_hi, P2, 1)):
                    t2s[hx] = pd.tile(
                        [128, BH, D], bf16, tag=f"t2{hx}", name=f"t2{hx}"
                    )
                    nc.sync.dma_start(t2s[hx][0:cn], src[:])
                for c0, cn, o1sb, rcol, hx in (
                    (0, 128, o1sb_lo, 0, 0), (128, P2, o1sb_hi, BH, 1),
                ):
                    t2 = t2s[hx]
                    addf = pd.tile([128, BH, D], bf16, tag=f"addf{hx}")
                    nc.vector.tensor_add(
                        addf[0:cn], t2[0:cn], o1sb[0:cn, :, 0:64]
                    )
                    res = pd.tile([128, BH, D], bf16, tag=f"res{hx}")
                    for bh in range(BH):
                        sc = recips[0:cn, rcol + bh : rcol + bh + 1]
                        if bh % 3 != 2:
                            nc.vector.tensor_scalar_mul(
                                res[0:cn, bh, :], addf[0:cn, bh, :], sc
                            )
                        else:
                            nc.scalar.mul(res[0:cn, bh, :], addf[0:cn, bh, :], sc)
                    for b in range(BSH):
                        nc.sync.dma_start(
                            out[b, c0 : c0 + cn, :],
                            res[0:cn, b * H : (b + 1) * H, :],
                        )

    nc.finalize()
    return nc


def _get_module():
    global _CACHED
    if _CACHED is None:
        _CACHED = _build_module()
    return _CACHED


def kernel(x, k_table_v, k_table_h, v_table_v, v_table_h, _trace=False):
    global LAST_EXEC_NS, LAST_TRACE
    from concourse.bass_utils import run_bass_kernel_spmd

    x = np.asarray(x, dtype=np.float32)
    iv, ih = _rel_indices()
    rel_k = np.asarray(k_table_v)[iv] + np.asarray(k_table_h)[ih]  # [N,N,D]
    rel_v = np.asarray(v_table_v)[iv] + np.asarray(v_table_h)[ih]  # [N,N,D]

    qkv = x.reshape(B, N, 3, H, D).transpose(2, 0, 3, 1, 4)  # [3,B,H,N,D]
    q, k, v = qkv[0], qkv[1], qkv[2]  # [B,H,N,D]

    rkT_host = np.zeros((D, N, NP), dtype=_bf16)  # [D,N(i),N(j)] j-padded
    rkT_host[:, :, :N] = rel_k.transpose(2, 0, 1).astype(_bf16)
    rv_host = np.ascontiguousarray(
        rel_v.transpose(1, 0, 2).astype(_bf16)
    )  # [N(j),N(i),D]

    def _padN(a):  # [D,BH,N] -> [D,BH,NP]
        out_ = np.zeros((D, BH, NP), dtype=_bf16)
        out_[:, :, :N] = a
        return out_

    in_maps = []
    for c in range(NCORES):
        qs = q[c * BSH : (c + 1) * BSH].reshape(BH, N, D)   # [BH,N,D]
        ks = k[c * BSH : (c + 1) * BSH].reshape(BH, N, D)
        vs = v[c * BSH : (c + 1) * BSH].reshape(BH, N, D)
        in_maps.append(
            {
                "qT2": _padN(qs.transpose(2, 0, 1).astype(_bf16)),  # [D,BH,NP]
                "kT2": _padN(ks.transpose(2, 0, 1).astype(_bf16)),
                "vb2": np.ascontiguousarray(
                    vs.transpose(1, 0, 2).astype(_bf16)
                ),  # [N,BH,D]
                "qTi": np.ascontiguousarray(
                    qs.transpose(2, 1, 0).astype(_bf16)
                ),  # [D,N,BH]
                "rkT": rkT_host,
                "rv": rv_host,
            }
        )

    nc = _get_module()
    res = run_bass_kernel_spmd(
        nc, in_maps, core_ids=list(range(NCORES)), trace=_trace
    )
    LAST_EXEC_NS = res.exec_time_ns
    LAST_TRACE = res.instructions_and_trace
    outs = [res.results[c]["out"].astype(np.float32) for c in range(NCORES)]
    return np.concatenate(outs, axis=0)
